# revision 1
# baseline (speedup 1.0000x reference)
"""Trainium2 Bass kernel for LinearSelfAttention (MobileViT-style).

Reference computation (per batch b, pixel p, patch n, channels c/o):
    qkv  = w_qkv @ x + b_qkv          # [B, 2C+1, P, N]
    q    = qkv[:, 0]                  # [B, P, N]
    key  = qkv[:, 1:1+C]
    val  = qkv[:, 1+C:]
    s    = softmax(q, axis=n)
    cv   = sum_n s * key              # [B, C, P]
    out  = w_out @ (relu(val) * cv[..., None]) + b_out

Strategy: data-parallel over B across 8 cores (2 batches each). Per core,
channels live on SBUF partitions; spatial (b, p, n) is the matmul moving
dim, processed in chunks of 512 (= 2 full p-rows, so softmax/reduce over
n stays chunk-local). All matmuls run in float32r (full-rate PE, ~1e-4
rel err). Softmax skips the max-subtraction (|q| < ~5, exp is safe in
fp32). The key bias folds out of the weighted sum (sum_n s == 1), so
cv = (sum_n exp(q)*key0)/Z + b_k, computed with a fused DVE
tensor_tensor_reduce against a PE-broadcast copy of exp(q).
"""

import numpy as np

import concourse.bass as bass
import concourse.mybir as mybir
import concourse.tile as tile
from concourse import bacc
from concourse.bass_utils import run_bass_kernel_spmd

B, C, P, N = 16, 512, 64, 256
O1 = 2 * C + 1
NCORES = 8
BPC = B // NCORES          # batches per core
S = BPC * P * N            # spatial per core = 32768
SCH = 512                  # chunk = 2 p-rows
PCH = SCH // N             # p-rows per chunk = 2
NCH = S // SCH             # 64 chunks
CT = C // 128              # 4 channel tiles

F32 = mybir.dt.float32
FR = mybir.dt.float32r
AX = mybir.AxisListType
ALU = mybir.AluOpType
ACT = mybir.ActivationFunctionType


def build():
    nc = bacc.Bacc("TRN2", target_bir_lowering=False, debug=False)

    x_d = nc.dram_tensor("x", [BPC, C, P, N], FR, kind="ExternalInput")
    w1_d = nc.dram_tensor("w1", [C, O1], FR, kind="ExternalInput")
    w1q_d = nc.dram_tensor("w1q", [C, 128], FR, kind="ExternalInput")
    w2_d = nc.dram_tensor("w2", [C, C], FR, kind="ExternalInput")
    bk_d = nc.dram_tensor("bk", [128, CT], F32, kind="ExternalInput")
    bv_d = nc.dram_tensor("bv", [128, CT], F32, kind="ExternalInput")
    bo_d = nc.dram_tensor("bo", [128, CT], F32, kind="ExternalInput")
    y_d = nc.dram_tensor("y", [BPC, C, P, N], F32, kind="ExternalOutput")

    with tile.TileContext(nc) as tc:
        with (
            tc.tile_pool(name="wp", bufs=1) as wp,
            tc.tile_pool(name="xp", bufs=3) as xp,
            tc.tile_pool(name="eqbp", bufs=2) as eqbp,
            tc.tile_pool(name="smallp", bufs=4) as smallp,
            tc.tile_pool(name="scrp", bufs=4) as scrp,
            tc.tile_pool(name="rvp", bufs=2) as rvp,
            tc.tile_pool(name="rsp", bufs=3) as rsp,
            tc.tile_pool(name="yop", bufs=2) as yop,
            tc.tile_pool(name="xsp", bufs=3) as xsp,
            tc.tile_pool(name="psqb", bufs=2, space="PSUM") as psqb,
            tc.tile_pool(name="pscv", bufs=2, space="PSUM") as pscv,
            tc.tile_pool(name="psv", bufs=2, space="PSUM") as psv,
            tc.tile_pool(name="psmm2", bufs=2, space="PSUM") as psmm2,
        ):
            # --- weights / constants, resident ---
            w1_t = []
            w1q_t = []
            w2_t = []
            for ct in range(CT):
                w1t = wp.tile([128, O1], FR, name=f"w1_{ct}")
                nc.sync.dma_start(out=w1t, in_=w1_d[ct * 128:(ct + 1) * 128, :])
                w1_t.append(w1t)
                w1qt = wp.tile([128, 128], FR, name=f"w1q_{ct}")
                nc.sync.dma_start(
                    out=w1qt, in_=w1q_d[ct * 128:(ct + 1) * 128, :]
                )
                w1q_t.append(w1qt)
                w2t = wp.tile([128, C], FR, name=f"w2_{ct}")
                nc.sync.dma_start(out=w2t, in_=w2_d[ct * 128:(ct + 1) * 128, :])
                w2_t.append(w2t)
            bk_t = wp.tile([128, CT], F32, name="bk_t")
            nc.sync.dma_start(out=bk_t, in_=bk_d[:, :])
            bv_t = wp.tile([128, CT], F32, name="bv_t")
            nc.sync.dma_start(out=bv_t, in_=bv_d[:, :])
            bo_t = wp.tile([128, CT], F32, name="bo_t")
            nc.sync.dma_start(out=bo_t, in_=bo_d[:, :])

            prev_tail = None

            def emit_tail(rs_list, bidx, p0):
                # mm2: y[o2, s] = sum_c w2T[c, o2] * rs[c, s], + b_out, DMA out
                for j in range(CT):
                    mm2_ps = psmm2.tile([128, SCH], F32, name=f"mm2_{j}",
                                        tag="mm2")
                    for i in range(CT):
                        nc.tensor.matmul(
                            mm2_ps,
                            w2_t[i][:, j * 128:(j + 1) * 128],
                            rs_list[i].rearrange("c p n -> c (p n)"),
                            start=(i == 0),
                            stop=(i == CT - 1),
                        )
                    yo = yop.tile([128, PCH, N], F32, name=f"yo_{j}")
                    nc.scalar.activation(
                        yo,
                        mm2_ps.rearrange("c (p n) -> c p n", p=PCH),
                        ACT.Identity,
                        bias=bo_t[:, j:j + 1],
                    )
                    nc.sync.dma_start(
                        out=y_d[bidx, j * 128:(j + 1) * 128, p0:p0 + PCH, :],
                        in_=yo,
                    )

            for ch in range(NCH):
                bidx = ch // (P // PCH)
                p0 = (ch % (P // PCH)) * PCH

                # --- x in ---
                xt = []
                for ct in range(CT):
                    t = xp.tile([128, PCH, N], FR, name=f"xt_{ct}")
                    nc.sync.dma_start(
                        out=t,
                        in_=x_d[bidx, ct * 128:(ct + 1) * 128, p0:p0 + PCH, :],
                    )
                    xt.append(t)
                xf = [t.rearrange("c p n -> c (p n)") for t in xt]

                # --- q, already broadcast to 128 partitions via the
                # rank-1 replicated weight trick: qb[c, s] = q[s] ---
                qb_ps = psqb.tile([128, SCH], F32, name="qb_ps", tag="qb")
                for ct in range(CT):
                    nc.tensor.matmul(
                        qb_ps, w1q_t[ct], xf[ct],
                        start=(ct == 0), stop=(ct == CT - 1),
                    )
                eqb_sb = eqbp.tile([128, SCH], F32, name="eqb_sb")
                nc.scalar.activation(eqb_sb, qb_ps, ACT.Exp)
                eqb3 = eqb_sb.rearrange("c (p n) -> c p n", p=PCH)

                # --- Z = sum_n exp(q) (per partition copy), rZ = 1/Z ---
                z_sb = smallp.tile([128, PCH], F32, name="z_sb")
                nc.vector.reduce_sum(z_sb, eqb3, axis=AX.X)
                rz_sb = smallp.tile([128, PCH], F32, name="rz_sb")
                nc.vector.reciprocal(rz_sb, z_sb)

                # --- xs = sum_n exp(q)*x  (weighted sum commutes with the
                # linear key projection: cv = W_k @ xs / Z + b_k) ---
                xs_t = xsp.tile([128, CT, PCH], FR, name="xs_t")
                for ct in range(CT):
                    x3 = xt[ct].bitcast(F32)
                    for p in range(PCH):
                        scr = scrp.tile([128, N], F32, name="scr", tag="scr")
                        nc.vector.scalar_tensor_tensor(
                            out=scr,
                            in0=x3[:, p, :],
                            scalar=1.0,
                            in1=eqb3[:, p, :],
                            op0=ALU.mult,
                            op1=ALU.mult,
                            accum_out=xs_t[:, ct, p:p + 1],
                        )

                # --- value tiles: relu(v + b_v) on ACT ---
                rv_list = []
                for i in range(CT):
                    vp = psv.tile([128, SCH], F32, name=f"v_ps{i}", tag="v")
                    for ct in range(CT):
                        nc.tensor.matmul(
                            vp,
                            w1_t[ct][:, C + i * 128:C + (i + 1) * 128],
                            xf[ct],
                            start=(ct == 0), stop=(ct == CT - 1),
                        )
                    rv = rvp.tile([128, PCH, N], F32, name=f"rv_{i}")
                    nc.scalar.activation(
                        rv,
                        vp.rearrange("c (p n) -> c p n", p=PCH),
                        ACT.Relu,
                        bias=bv_t[:, i:i + 1],
                    )
                    rv_list.append(rv)

                # --- previous chunk's output projection (keeps PE ahead of
                # the DVE-dependent cv matmuls below) ---
                if prev_tail is not None:
                    emit_tail(*prev_tail)

                # --- cv via tiny matmuls: cv[o,p] = sum_c W_kT[c,o] xs[c,p]
                cv_ps = pscv.tile([128, CT * PCH], F32, name="cv_ps")
                for i in range(CT):
                    for ct in range(CT):
                        nc.tensor.matmul(
                            cv_ps[:, i * PCH:(i + 1) * PCH],
                            w1_t[ct][:, i * 128:(i + 1) * 128],
                            xs_t[:, ct, :],
                            start=(ct == 0), stop=(ct == CT - 1),
                        )
                rs_list = []
                for i in range(CT):
                    cvni = smallp.tile([128, PCH], F32, name=f"cvn_{i}")
                    nc.vector.tensor_mul(
                        cvni, cv_ps[:, i * PCH:(i + 1) * PCH], rz_sb
                    )
                    nc.vector.tensor_scalar_add(cvni, cvni, bk_t[:, i:i + 1])
                    rs = rsp.tile([128, PCH, N], FR, name=f"rs_{i}")
                    nc.vector.tensor_mul(
                        rs, rv_list[i], cvni.to_broadcast((128, PCH, N))
                    )
                    rs_list.append(rs)
                prev_tail = (rs_list, bidx, p0)

            emit_tail(*prev_tail)

    nc.compile()
    return nc


_NC = None


def _get_nc():
    global _NC
    if _NC is None:
        _NC = build()
    return _NC


def _prep_inputs(x, w_qkv, b_qkv, w_out, b_out):
    x = np.ascontiguousarray(np.asarray(x, dtype=np.float32))
    w_qkv = np.asarray(w_qkv, dtype=np.float32)
    b_qkv = np.asarray(b_qkv, dtype=np.float32)
    w_out = np.asarray(w_out, dtype=np.float32)
    b_out = np.asarray(b_out, dtype=np.float32)

    # permute qkv output channels to [key(512), value(512), q(1)]
    perm = np.concatenate([np.arange(1, 1 + C), np.arange(1 + C, O1), [0]])
    w1 = np.ascontiguousarray(w_qkv[perm].T)          # [C, O1]
    # q weight column replicated across 128 output partitions (rank-1
    # broadcast trick: (1 w_q^T)^T @ x = broadcast of q over partitions)
    w1q = np.ascontiguousarray(np.repeat(w_qkv[0][:, None], 128, axis=1))
    w2 = np.ascontiguousarray(w_out.T)                # [C, C]
    bk = np.ascontiguousarray(b_qkv[1:1 + C].reshape(CT, 128).T)
    bv = np.ascontiguousarray(b_qkv[1 + C:].reshape(CT, 128).T)
    bo = np.ascontiguousarray(b_out.reshape(CT, 128).T)

    shared = {"w1": w1, "w1q": w1q, "w2": w2, "bk": bk, "bv": bv, "bo": bo}
    in_maps = [
        {"x": np.ascontiguousarray(x[i * BPC:(i + 1) * BPC]), **shared}
        for i in range(NCORES)
    ]
    return in_maps


def run(in_maps, trace=False, **kwargs):
    nc = _get_nc()
    return run_bass_kernel_spmd(
        nc, in_maps, core_ids=list(range(NCORES)), trace=trace, **kwargs
    )


def kernel(x, w_qkv, b_qkv, w_out, b_out):
    in_maps = _prep_inputs(x, w_qkv, b_qkv, w_out, b_out)
    res = run(in_maps)
    return np.concatenate([r["y"] for r in res.results], axis=0)



# revision 4
# speedup vs baseline: 1.0691x; 1.0691x over previous
"""Trainium2 Bass kernel for LinearSelfAttention (MobileViT-style).

Reference computation (per batch b, pixel p, patch n, channels c/o):
    qkv  = w_qkv @ x + b_qkv          # [B, 2C+1, P, N]
    q    = qkv[:, 0]                  # [B, P, N]
    key  = qkv[:, 1:1+C]
    val  = qkv[:, 1+C:]
    s    = softmax(q, axis=n)
    cv   = sum_n s * key              # [B, C, P]
    out  = w_out @ (relu(val) * cv[..., None]) + b_out

Strategy: data-parallel over B across 8 cores (2 batches each). Per core,
channels live on SBUF partitions; spatial (b, p, n) is the matmul moving
dim, processed in chunks of 512 (= 2 full p-rows, so softmax/reduce over
n stays chunk-local). All matmuls run in bf16 (full-rate PE + FWL hides
LDWEIGHTS; inputs/outputs are cast host-side). Softmax skips the
max-subtraction (|q| < ~5, exp is safe). The key bias folds out of the
weighted sum (sum_n s == 1), so cv = W_k @ (sum_n exp(q)*x)/Z + b_k,
with the weighted x-sum done on DVE against a PE-broadcast copy of
exp(q). The tiny cv matmuls are batched over groups of 4 chunks so their
LDWEIGHTS overhead amortizes.
"""

import numpy as np

import concourse.bass as bass
import concourse.mybir as mybir
import concourse.tile as tile
from concourse import bacc
from concourse.bass_utils import run_bass_kernel_spmd

B, C, P, N = 16, 512, 64, 256
NCORES = 8
BPC = B // NCORES          # batches per core
S = BPC * P * N            # spatial per core = 32768
SCH = 512                  # chunk = 2 p-rows
PCH = SCH // N             # p-rows per chunk = 2
NCH = S // SCH             # 64 chunks
CT = C // 128              # 4 channel tiles
GRP = 4                    # chunks per cv-batch group
NG = NCH // GRP            # 16 groups
GP = GRP * PCH             # p-rows per group = 8

F32 = mybir.dt.float32
BF16 = mybir.dt.bfloat16
AX = mybir.AxisListType
ALU = mybir.AluOpType
ACT = mybir.ActivationFunctionType


def build():
    nc = bacc.Bacc("TRN2", target_bir_lowering=False, debug=False)

    x_d = nc.dram_tensor("x", [BPC, C, P, N], BF16, kind="ExternalInput")
    # w1: [C, 1024] = [key cols | value cols]
    w1_d = nc.dram_tensor("w1", [C, 2 * C], BF16, kind="ExternalInput")
    w1q_d = nc.dram_tensor("w1q", [C, 128], BF16, kind="ExternalInput")
    w2_d = nc.dram_tensor("w2", [C, C], BF16, kind="ExternalInput")
    bk_d = nc.dram_tensor("bk", [128, CT], F32, kind="ExternalInput")
    bv_d = nc.dram_tensor("bv", [128, CT], F32, kind="ExternalInput")
    bo_d = nc.dram_tensor("bo", [128, CT], F32, kind="ExternalInput")
    y_d = nc.dram_tensor("y", [BPC, C, P, N], BF16, kind="ExternalOutput")

    with tile.TileContext(nc) as tc:
        with (
            tc.tile_pool(name="wp", bufs=1) as wp,
            tc.tile_pool(name="xp", bufs=3) as xp,
            tc.tile_pool(name="eqbp", bufs=3) as eqbp,
            tc.tile_pool(name="smallp", bufs=4) as smallp,
            tc.tile_pool(name="scrp", bufs=4) as scrp,
            tc.tile_pool(name="rvp", bufs=6) as rvp,
            tc.tile_pool(name="rsp", bufs=2) as rsp,
            tc.tile_pool(name="yop", bufs=2) as yop,
            tc.tile_pool(name="grpp", bufs=2) as grpp,
            tc.tile_pool(name="psqb", bufs=2, space="PSUM") as psqb,
            tc.tile_pool(name="pscv", bufs=2, space="PSUM") as pscv,
            tc.tile_pool(name="psv", bufs=2, space="PSUM") as psv,
            tc.tile_pool(name="psmm2", bufs=2, space="PSUM") as psmm2,
        ):
            # --- weights / constants, resident ---
            w1_t = []
            w1q_t = []
            w2_t = []
            for ct in range(CT):
                w1t = wp.tile([128, 2 * C], BF16, name=f"w1_{ct}")
                nc.sync.dma_start(out=w1t, in_=w1_d[ct * 128:(ct + 1) * 128, :])
                w1_t.append(w1t)
                w1qt = wp.tile([128, 128], BF16, name=f"w1q_{ct}")
                nc.sync.dma_start(
                    out=w1qt, in_=w1q_d[ct * 128:(ct + 1) * 128, :]
                )
                w1q_t.append(w1qt)
                w2t = wp.tile([128, C], BF16, name=f"w2_{ct}")
                nc.sync.dma_start(out=w2t, in_=w2_d[ct * 128:(ct + 1) * 128, :])
                w2_t.append(w2t)
            bk_t = wp.tile([128, CT], F32, name="bk_t")
            nc.sync.dma_start(out=bk_t, in_=bk_d[:, :])
            bv_t = wp.tile([128, CT], F32, name="bv_t")
            nc.sync.dma_start(out=bv_t, in_=bv_d[:, :])
            bo_t = wp.tile([128, CT], F32, name="bo_t")
            nc.sync.dma_start(out=bo_t, in_=bo_d[:, :])

            # per-group state: rv tiles for 4 chunks + cvn, indexed [g % 2]
            rv_hist = {}        # (ch, i) -> rv tile
            cvn_hist = {}       # g -> cvn tile [128, CT, GP] bf16

            def emit_tail(ch):
                # rs = rv * cvn (DVE), mm2: y = w2T @ rs (+ b_out), DMA out
                g, j = divmod(ch, GRP)
                bidx = ch // (P // PCH)
                p0 = (ch % (P // PCH)) * PCH
                cvn = cvn_hist[g]
                rs_list = []
                for i in range(CT):
                    rv = rv_hist.pop((ch, i))
                    rs = rsp.tile([128, PCH, N], BF16, name=f"rs_{i}",
                                  tag=f"rs_{i}")
                    nc.vector.tensor_mul(
                        rs, rv,
                        cvn[:, i, j * PCH:(j + 1) * PCH].to_broadcast(
                            (128, PCH, N)),
                    )
                    rs_list.append(rs)
                for k in range(CT):
                    mm2_ps = psmm2.tile([128, SCH], F32, name=f"mm2_{k}",
                                        tag="mm2")
                    for i in range(CT):
                        nc.tensor.matmul(
                            mm2_ps,
                            w2_t[i][:, k * 128:(k + 1) * 128],
                            rs_list[i].rearrange("c p n -> c (p n)"),
                            start=(i == 0),
                            stop=(i == CT - 1),
                        )
                    yo = yop.tile([128, PCH, N], BF16, name=f"yo_{k}",
                                  tag=f"yo_{k}")
                    mm2_3 = mm2_ps.rearrange("c (p n) -> c p n", p=PCH)
                    if k < 2:
                        # scalar engine: out = identity(in + bias)
                        nc.scalar.activation(
                            yo, mm2_3, ACT.Identity, bias=bo_t[:, k:k + 1],
                        )
                    else:
                        # vector engine: out = in + bias
                        nc.vector.tensor_scalar_add(
                            yo, mm2_3, bo_t[:, k:k + 1],
                        )
                    nc.sync.dma_start(
                        out=y_d[bidx, k * 128:(k + 1) * 128, p0:p0 + PCH, :],
                        in_=yo,
                    )

            for ch in range(NCH):
                g, j = divmod(ch, GRP)
                bidx = ch // (P // PCH)
                p0 = (ch % (P // PCH)) * PCH

                if j == 0:
                    xs_g = grpp.tile([128, CT, GP], F32, name="xs_g",
                                     tag="xs_g")
                    rz_g = grpp.tile([128, GP], F32, name="rz_g", tag="rz_g")

                # --- x in (bf16) ---
                xt = []
                for ct in range(CT):
                    t = xp.tile([128, PCH, N], BF16, name=f"xt_{ct}")
                    nc.sync.dma_start(
                        out=t,
                        in_=x_d[bidx, ct * 128:(ct + 1) * 128, p0:p0 + PCH, :],
                    )
                    xt.append(t)
                xf = [t.rearrange("c p n -> c (p n)") for t in xt]

                # --- q broadcast to 128 partitions via replicated weights ---
                qb_ps = psqb.tile([128, SCH], F32, name="qb_ps", tag="qb")
                for ct in range(CT):
                    nc.tensor.matmul(
                        qb_ps, w1q_t[ct], xf[ct],
                        start=(ct == 0), stop=(ct == CT - 1),
                    )
                eqb_sb = eqbp.tile([128, PCH, N], BF16, name="eqb_sb")
                nc.scalar.activation(
                    eqb_sb, qb_ps.rearrange("c (p n) -> c p n", p=PCH),
                    ACT.Exp,
                )

                # --- Z = sum_n exp(q) (per partition copy), rZ = 1/Z ---
                z_sb = smallp.tile([128, PCH], F32, name="z_sb")
                nc.vector.reduce_sum(z_sb, eqb_sb, axis=AX.X)
                nc.vector.reciprocal(rz_g[:, j * PCH:(j + 1) * PCH], z_sb)

                # --- xs = sum_n exp(q)*x into group accumulator ---
                for ct in range(CT):
                    for p in range(PCH):
                        scr = scrp.tile([128, N], BF16, name="scr", tag="scr")
                        nc.vector.scalar_tensor_tensor(
                            out=scr,
                            in0=xt[ct][:, p, :],
                            scalar=1.0,
                            in1=eqb_sb[:, p, :],
                            op0=ALU.mult,
                            op1=ALU.mult,
                            accum_out=xs_g[:, ct, j * PCH + p:j * PCH + p + 1],
                        )

                # --- value tiles: relu(v + b_v) on ACT ---
                for i in range(CT):
                    vp = psv.tile([128, SCH], F32, name=f"v_ps{i}", tag="v")
                    for ct in range(CT):
                        nc.tensor.matmul(
                            vp,
                            w1_t[ct][:, C + i * 128:C + (i + 1) * 128],
                            xf[ct],
                            start=(ct == 0), stop=(ct == CT - 1),
                        )
                    rv = rvp.tile([128, PCH, N], BF16, name=f"rv_{i}")
                    nc.scalar.activation(
                        rv,
                        vp.rearrange("c (p n) -> c p n", p=PCH),
                        ACT.Relu,
                        bias=bv_t[:, i:i + 1],
                    )
                    rv_hist[(ch, i)] = rv

                # --- previous group's chunk: rs + output projection ---
                if ch >= GRP:
                    emit_tail(ch - GRP)

                # --- end of group: batched cv matmuls + cvn ---
                if j == GRP - 1:
                    xsb = grpp.tile([128, CT, GP], BF16, name="xsb",
                                    tag="xsb")
                    nc.vector.tensor_copy(xsb, xs_g)
                    cv_ps = pscv.tile([128, CT * GP], F32, name="cv_ps",
                                      tag="cv")
                    for i in range(CT):
                        for ct in range(CT):
                            nc.tensor.matmul(
                                cv_ps[:, i * GP:(i + 1) * GP],
                                w1_t[ct][:, i * 128:(i + 1) * 128],
                                xsb[:, ct, :],
                                start=(ct == 0), stop=(ct == CT - 1),
                            )
                    cvn = grpp.tile([128, CT, GP], BF16, name="cvn",
                                    tag="cvn")
                    for i in range(CT):
                        nc.vector.tensor_mul(
                            cvn[:, i, :], cv_ps[:, i * GP:(i + 1) * GP], rz_g
                        )
                        nc.vector.tensor_scalar_add(
                            cvn[:, i, :], cvn[:, i, :], bk_t[:, i:i + 1]
                        )
                    cvn_hist[g] = cvn
                    if g >= 2:
                        cvn_hist.pop(g - 2, None)

            for ch in range(NCH - GRP, NCH):
                emit_tail(ch)

    nc.compile()
    return nc


_NC = None


def _get_nc():
    global _NC
    if _NC is None:
        _NC = build()
    return _NC


def _prep_inputs(x, w_qkv, b_qkv, w_out, b_out):
    import ml_dtypes

    bf16 = ml_dtypes.bfloat16
    x = np.asarray(x, dtype=np.float32)
    w_qkv = np.asarray(w_qkv, dtype=np.float32)
    b_qkv = np.asarray(b_qkv, dtype=np.float32)
    w_out = np.asarray(w_out, dtype=np.float32)
    b_out = np.asarray(b_out, dtype=np.float32)

    # [C, 1024] = [keyT | valueT]
    w1 = np.ascontiguousarray(
        np.concatenate([w_qkv[1:1 + C].T, w_qkv[1 + C:].T], axis=1)
    ).astype(bf16)
    # q weight column replicated across 128 output partitions (rank-1
    # broadcast trick: (1 w_q^T)^T @ x = broadcast of q over partitions)
    w1q = np.ascontiguousarray(
        np.repeat(w_qkv[0][:, None], 128, axis=1)
    ).astype(bf16)
    w2 = np.ascontiguousarray(w_out.T).astype(bf16)
    bk = np.ascontiguousarray(b_qkv[1:1 + C].reshape(CT, 128).T)
    bv = np.ascontiguousarray(b_qkv[1 + C:].reshape(CT, 128).T)
    bo = np.ascontiguousarray(b_out.reshape(CT, 128).T)

    xb = x.astype(bf16)
    shared = {"w1": w1, "w1q": w1q, "w2": w2, "bk": bk, "bv": bv, "bo": bo}
    in_maps = [
        {"x": np.ascontiguousarray(xb[i * BPC:(i + 1) * BPC]), **shared}
        for i in range(NCORES)
    ]
    return in_maps


def run(in_maps, trace=False, **kwargs):
    nc = _get_nc()
    return run_bass_kernel_spmd(
        nc, in_maps, core_ids=list(range(NCORES)), trace=trace, **kwargs
    )


def kernel(x, w_qkv, b_qkv, w_out, b_out):
    in_maps = _prep_inputs(x, w_qkv, b_qkv, w_out, b_out)
    res = run(in_maps)
    return np.concatenate(
        [np.asarray(r["y"]).astype(np.float32) for r in res.results], axis=0
    )
